# revision 1
# baseline (speedup 1.0000x reference)
"""Trainium2 Bass kernel for nn_DifferentiableBundleAdjustment.

Reference semantics (see problem):
  delta = dba_params[..., :7] * 0.1
  pos   = cumsum of delta[..., 0:3] along S, seeded with gt_state[:, 0, 0:3]
  quats = sequential q_t = normalize(q_{t-1} + delta[:, t-1, 3:7]), seeded
          with gt_state[:, 0, 3:7]
  out[..., 0:3] = pos, out[..., 3:7] = quats, out[..., 7:15] = 0

Sharding: pure data parallel over B=4096 across 8 cores (512 samples/core).
Per core:
  - 512 samples = 4 groups of 128; partition dim = sample-within-group.
  - pos: DVE tensor_tensor_scan (cumsum along free dim) per (group, comp).
  - quats: two software-pipelined chains (2 groups each), per step:
      u = 0.1*dq + q   (DVE scalar_tensor_tensor)
      n2 = sum(u^2)    (DVE tensor_mul + tensor_reduce)
      n = sqrt(n2)     (ACT)
      r = 1/n          (DVE reciprocal)
      q' = u*r         (DVE tensor_mul, broadcast r) -> written into outbuf
  - zeros: POOL memset of channels 7:15.
  - IO: contiguous chunked DMAs; output assembled in one SBUF buffer laid
    out exactly like out[b, s, ch] so the store is contiguous per partition.

This walrus build accepts at most one sync-wait per instruction; a
post-trace pass splits multi-wait instructions into single-wait NoOps.
"""

import numpy as np

import concourse.bass as bass
import concourse.mybir as mybir
from concourse.bass_utils import run_bass_kernel_spmd
from concourse.tile import TileContext

B, S, CH_IN, CH_OUT = 4096, 128, 32, 15
N_CORES = 8
BL = B // N_CORES  # 512 samples per core
G = BL // 128  # 4 groups
P = 128
F32 = mybir.dt.float32
Alu = mybir.AluOpType
X = mybir.AxisListType.X

_cache = {}


def _split_multi_waits(nc):
    """Walrus here allows only one sync-wait per instruction; hoist extras
    onto same-engine NoOps."""
    n = 0
    for fn in nc.m.functions:
        for blk in fn.blocks:
            out = []
            changed = False
            for inst in blk.instructions:
                si = getattr(inst, "sync_info", None)
                waits = list(si.on_wait) if si is not None and si.on_wait else []
                if len(waits) > 1:
                    for w in waits[:-1]:
                        nop = mybir.InstNoOp(name=f"{inst.name}-ws{n}")
                        nop.engine = inst.engine
                        nop.sync_info = mybir.SyncInfo(on_wait=[w], on_update=[])
                        out.append(nop)
                        n += 1
                    si.on_wait = [waits[-1]]
                    changed = True
                out.append(inst)
            if changed:
                blk.instructions = out
    return n


def _build():
    """One SPMD program; every core runs the same kernel on its slice."""
    nc = bass.Bass()
    dq_d = nc.dram_tensor("dq8", [BL, S, 8], F32, kind="ExternalInput")
    init_d = nc.dram_tensor("init8", [BL, 8], F32, kind="ExternalInput")
    out_d = nc.dram_tensor("out", [BL, S, CH_OUT], F32, kind="ExternalOutput")

    NCHUNK = 4  # input staging chunks along S
    SC = S // NCHUNK
    NOUT = 4  # output chunks along S
    OC = S // NOUT

    with TileContext(nc) as tc:
        with tc.tile_pool(name="pool", bufs=1) as pool, tc.tile_pool(
            name="w", bufs=6
        ) as wp:
            dq_t = pool.tile([P, G, S, 8], F32)
            outb = pool.tile([P, G, S, CH_OUT], F32)
            init_t = pool.tile([P, G, 8], F32)

            dq_r = dq_d.rearrange("(g p) s c -> p g s c", g=G)
            for k in range(NCHUNK):
                sl = slice(k * SC, (k + 1) * SC)
                nc.sync.dma_start(out=dq_t[:, :, sl, :], in_=dq_r[:, :, sl, :])
            nc.sync.dma_start(
                out=init_t, in_=init_d.rearrange("(g p) c -> p g c", g=G)
            )

            # zeros for channels 7:15 (POOL, off the critical path)
            nc.gpsimd.memset(outb[:, :, :, 7:15], 0.0)

            # init pos/quat into s=0
            nc.gpsimd.tensor_copy(out=outb[:, :, 0, 0:3], in_=init_t[:, :, 0:3])
            nc.vector.tensor_copy(out=outb[:, :, 0, 3:7], in_=init_t[:, :, 3:7])

            # --- positions: scaled cumsum via tensor_tensor_scan ------------
            # prescale pos deltas by 0.1 on POOL (in-place, strided)
            for k in range(NCHUNK):
                sl = slice(k * SC, (k + 1) * SC)
                nc.gpsimd.tensor_scalar_mul(
                    out=dq_t[:, :, sl, 0:3], in0=dq_t[:, :, sl, 0:3], scalar1=0.1
                )
            for g in range(G):
                for c in range(3):
                    nc.vector.tensor_tensor_scan(
                        out=outb[:, g, 1:S, c],
                        data0=dq_t[:, g, 0 : S - 1, c],
                        data1=dq_t[:, g, 0 : S - 1, c],
                        initial=init_t[:, g, c : c + 1],
                        op0=Alu.add,
                        op1=Alu.bypass,
                    )

            # --- quaternions: 2 software-pipelined chains -------------------
            GS = {0: slice(0, 2), 1: slice(2, 4)}
            us, ns = {}, {}

            def p1(c, t):
                gs = GS[c]
                u = wp.tile([P, 2, 4], F32, name=f"u{c}")
                sq = wp.tile([P, 2, 4], F32, name=f"s{c}")
                n2 = wp.tile([P, 2], F32, name=f"n2{c}")
                n = wp.tile([P, 2], F32, name=f"n{c}")
                # u = 0.1*dq + q_prev   (dq columns 3:7 are raw deltas)
                nc.vector.scalar_tensor_tensor(
                    out=u,
                    in0=dq_t[:, gs, t - 1, 3:7],
                    scalar=0.1,
                    in1=outb[:, gs, t - 1, 3:7],
                    op0=Alu.mult,
                    op1=Alu.add,
                )
                nc.vector.tensor_mul(out=sq, in0=u, in1=u)
                nc.vector.tensor_reduce(out=n2, in_=sq, axis=X, op=Alu.add)
                nc.scalar.sqrt(out=n, in_=n2)
                us[c], ns[c] = u, n

            def p2(c, t):
                gs = GS[c]
                r = wp.tile([P, 2], F32, name=f"r{c}")
                nc.vector.reciprocal(out=r, in_=ns[c])
                rb = r.unsqueeze(-1).broadcast_to([P, 2, 4])
                nc.vector.tensor_mul(out=outb[:, gs, t, 3:7], in0=us[c], in1=rb)

            p1(0, 1)
            for t in range(1, S):
                p1(1, t)
                p2(0, t)
                if t + 1 < S:
                    p1(0, t + 1)
                p2(1, t)

            # --- store: contiguous per-partition chunks ---------------------
            out_r = out_d.rearrange("(g p) s c -> p g s c", g=G)
            for k in range(NOUT):
                sl = slice(k * OC, (k + 1) * OC)
                for g in range(G):
                    nc.sync.dma_start(
                        out=out_r[:, g, sl, :], in_=outb[:, g, sl, :]
                    )
    _split_multi_waits(nc)
    return nc


def kernel(dba_params, imu_measurements=None, gt_state=None, **_):
    dba_params = np.asarray(dba_params)
    gt_state = np.asarray(gt_state)
    assert dba_params.shape == (B, S, CH_IN), dba_params.shape

    if "nc" not in _cache:
        _cache["nc"] = _build()
    nc = _cache["nc"]

    dq8 = np.ascontiguousarray(dba_params[:, :, 0:8], dtype=np.float32)
    init8 = np.zeros((B, 8), dtype=np.float32)
    init8[:, 0:7] = gt_state[:, 0, 0:7]

    in_maps = []
    for c in range(N_CORES):
        sl = slice(c * BL, (c + 1) * BL)
        in_maps.append(
            {"dq8": np.ascontiguousarray(dq8[sl]), "init8": np.ascontiguousarray(init8[sl])}
        )
    res = run_bass_kernel_spmd(nc, in_maps, core_ids=list(range(N_CORES)))
    out = np.empty((B, S, CH_OUT), dtype=np.float32)
    for c in range(N_CORES):
        out[c * BL : (c + 1) * BL] = res.results[c]["out"]
    return out


# revision 7
# speedup vs baseline: 1.3508x; 1.3508x over previous
"""Trainium2 Bass kernel for nn_DifferentiableBundleAdjustment.

Reference semantics (see problem):
  delta = dba_params[..., :7] * 0.1
  pos   = cumsum of delta[..., 0:3] along S, seeded with gt_state[:, 0, 0:3]
  quats = sequential q_t = normalize(q_{t-1} + delta[:, t-1, 3:7]), seeded
          with gt_state[:, 0, 3:7]
  out[..., 0:3] = pos, out[..., 3:7] = quats, out[..., 7:15] = 0

Sharding: pure data parallel over B=4096 across 8 cores (512 samples/core).
Per core:
  - 512 samples = 4 groups of 128; partition dim = sample-within-group.
  - pos: DVE tensor_tensor_scan (cumsum along free dim) per (group, comp).
  - quats: two software-pipelined chains (2 groups each), per step:
      u = 0.1*dq + q   (DVE scalar_tensor_tensor)
      n2 = sum(u^2)    (DVE tensor_mul + tensor_reduce)
      n = sqrt(n2)     (ACT)
      r = 1/n          (DVE reciprocal)
      q' = u*r         (DVE tensor_mul, broadcast r) -> written into outbuf
  - zeros: POOL memset of channels 7:15.
  - IO: contiguous chunked DMAs; output assembled in one SBUF buffer laid
    out exactly like out[b, s, ch] so the store is contiguous per partition.

This walrus build accepts at most one sync-wait per instruction; a
post-trace pass splits multi-wait instructions into single-wait NoOps.
"""

import numpy as np

import concourse.bass as bass
import concourse.mybir as mybir
from concourse.bass_utils import run_bass_kernel_spmd
from concourse.tile import TileContext

B, S, CH_IN, CH_OUT = 4096, 128, 32, 15
N_CORES = 8
BL = B // N_CORES  # 512 samples per core
G = BL // 128  # 4 groups
P = 128
F32 = mybir.dt.float32
Alu = mybir.AluOpType
X = mybir.AxisListType.X

_cache = {}


def _split_multi_waits(nc):
    """Walrus here allows only one sync-wait per instruction; hoist extras
    onto same-engine NoOps."""
    n = 0
    for fn in nc.m.functions:
        for blk in fn.blocks:
            out = []
            changed = False
            for inst in blk.instructions:
                si = getattr(inst, "sync_info", None)
                waits = list(si.on_wait) if si is not None and si.on_wait else []
                if len(waits) > 1:
                    for w in waits[:-1]:
                        nop = mybir.InstNoOp(name=f"{inst.name}-ws{n}")
                        nop.engine = inst.engine
                        nop.sync_info = mybir.SyncInfo(on_wait=[w], on_update=[])
                        out.append(nop)
                        n += 1
                    si.on_wait = [waits[-1]]
                    changed = True
                out.append(inst)
            if changed:
                blk.instructions = out
    return n


def _build():
    """One SPMD program; every core runs the same kernel on its slice."""
    nc = bass.Bass()
    dq_d = nc.dram_tensor("dq8", [BL, S, 8], F32, kind="ExternalInput")
    init_d = nc.dram_tensor("init8", [BL, 8], F32, kind="ExternalInput")
    out_d = nc.dram_tensor("out", [BL, S, CH_OUT], F32, kind="ExternalOutput")

    NCHUNK = 4  # input staging chunks along S
    SC = S // NCHUNK
    NOUT = 4  # output chunks along S
    OC = S // NOUT

    with TileContext(nc) as tc:
        with tc.tile_pool(name="pool", bufs=1) as pool, tc.tile_pool(
            name="w", bufs=6
        ) as wp:
            dq_t = pool.tile([P, G, S, 8], F32)
            outb = pool.tile([P, G, S, CH_OUT], F32)
            init_t = pool.tile([P, G, 8], F32)

            dq_r = dq_d.rearrange("(g p) s c -> p g s c", g=G)
            for k in range(NCHUNK):
                sl = slice(k * SC, (k + 1) * SC)
                nc.sync.dma_start(out=dq_t[:, :, sl, :], in_=dq_r[:, :, sl, :])
            nc.sync.dma_start(
                out=init_t, in_=init_d.rearrange("(g p) c -> p g c", g=G)
            )

            # zeros for channels 7:15 (POOL, off the critical path)
            nc.gpsimd.memset(outb[:, :, :, 7:15], 0.0)

            # init pos/quat into s=0
            nc.gpsimd.tensor_copy(out=outb[:, :, 0, 0:3], in_=init_t[:, :, 0:3])
            nc.vector.tensor_copy(out=outb[:, :, 0, 3:7], in_=init_t[:, :, 3:7])

            # --- positions: scaled cumsum via tensor_tensor_scan ------------
            # prescale pos deltas by 0.1 on POOL (in-place, strided)
            for k in range(NCHUNK):
                sl = slice(k * SC, (k + 1) * SC)
                nc.gpsimd.tensor_scalar_mul(
                    out=dq_t[:, :, sl, 0:3], in0=dq_t[:, :, sl, 0:3], scalar1=0.1
                )
            for g in range(G):
                for c in range(3):
                    nc.vector.tensor_tensor_scan(
                        out=outb[:, g, 1:S, c],
                        data0=dq_t[:, g, 0 : S - 1, c],
                        data1=dq_t[:, g, 0 : S - 1, c],
                        initial=init_t[:, g, c : c + 1],
                        op0=Alu.add,
                        op1=Alu.bypass,
                    )

            # --- quaternions: 4 chains (1 group each), pair-batched sqrt ----
            # chain state u_t kept unnormalized in ubuf; q_t = u_t * r_t is
            # reconstructed in a parallel postpass. Update is one fused STT:
            #   u_t = (u_{t-1} * r_{t-1}) + 0.1*dq_t  (dq prescaled by 0.1)
            # prescale quat deltas on POOL (pos cols already scaled above)
            for k in range(NCHUNK):
                sl = slice(k * SC, (k + 1) * SC)
                nc.gpsimd.tensor_scalar_mul(
                    out=dq_t[:, :, sl, 3:7], in0=dq_t[:, :, sl, 3:7], scalar1=0.1
                )

            # w-space recurrence: w_t = w_{t-1} + ||w_{t-1}|| * 0.1*dq_t, with
            # w_1 = q0 + 0.1*dq_1. Then q_t = w_t / ||w_t|| exactly. The serial
            # path per step is upd (STT) -> sum-of-squares (STT accum) ->
            # sqrt (ACT); no reciprocal in the loop.
            wbuf = [pool.tile([P, S, 4], F32, name=f"wbuf{g}") for g in range(G)]
            PAIRS = ((0, 1), (2, 3))
            # exact norms m_t = ||w_t||, one buffer per pair: [P, S, 2]
            nbufp = [pool.tile([P, S, 2], F32, name=f"nbufp{pi}") for pi in range(2)]

            def p1(pi, t):
                m2 = wp.tile([P, 2], F32, name=f"m2p{pi}")
                for a, g in enumerate(PAIRS[pi]):
                    if t == 1:
                        nc.vector.scalar_tensor_tensor(
                            out=wbuf[g][:, 1, :],
                            in0=init_t[:, g, 3:7],
                            scalar=1.0,
                            in1=dq_t[:, g, 0, 3:7],
                            op0=Alu.mult,
                            op1=Alu.add,
                        )
                    else:
                        nc.vector.scalar_tensor_tensor(
                            out=wbuf[g][:, t, :],
                            in0=dq_t[:, g, t - 1, 3:7],
                            scalar=nbufp[pi][:, t - 1, a : a + 1],
                            in1=wbuf[g][:, t - 1, :],
                            op0=Alu.mult,
                            op1=Alu.add,
                        )
                for a, g in enumerate(PAIRS[pi]):
                    sq = wp.tile([P, 4], F32, name=f"sqp{pi}{a}")
                    nc.vector.scalar_tensor_tensor(
                        out=sq,
                        in0=wbuf[g][:, t, :],
                        scalar=1.0,
                        in1=wbuf[g][:, t, :],
                        op0=Alu.mult,
                        op1=Alu.mult,
                        accum_out=m2[:, a : a + 1],
                    )
                nc.scalar.sqrt(out=nbufp[pi][:, t, :], in_=m2)

            p1(0, 1)
            for t in range(1, S):
                p1(1, t)
                if t + 1 < S:
                    p1(0, t + 1)

            # parallel postpass: q_t = w_t / m_t -> outbuf quat columns
            rp = [pool.tile([P, S, 2], F32, name=f"rp{pi}") for pi in range(2)]
            for pi in range(2):
                nc.vector.reciprocal(
                    out=rp[pi][:, 1:S, :], in_=nbufp[pi][:, 1:S, :]
                )
            for g in range(G):
                pi, a = (0, g) if g < 2 else (1, g - 2)
                rb = rp[pi][:, 1:S, a : a + 1].broadcast_to([P, S - 1, 4])
                nc.vector.tensor_mul(
                    out=outb[:, g, 1:S, 3:7], in0=wbuf[g][:, 1:S, :], in1=rb
                )

            # --- store: contiguous per-partition chunks ---------------------
            out_r = out_d.rearrange("(g p) s c -> p g s c", g=G)
            for k in range(NOUT):
                sl = slice(k * OC, (k + 1) * OC)
                for g in range(G):
                    nc.sync.dma_start(
                        out=out_r[:, g, sl, :], in_=outb[:, g, sl, :]
                    )
    _split_multi_waits(nc)
    return nc


def kernel(dba_params, imu_measurements=None, gt_state=None, **_):
    dba_params = np.asarray(dba_params)
    gt_state = np.asarray(gt_state)
    assert dba_params.shape == (B, S, CH_IN), dba_params.shape

    if "nc" not in _cache:
        _cache["nc"] = _build()
    nc = _cache["nc"]

    dq8 = np.ascontiguousarray(dba_params[:, :, 0:8], dtype=np.float32)
    init8 = np.zeros((B, 8), dtype=np.float32)
    init8[:, 0:7] = gt_state[:, 0, 0:7]

    in_maps = []
    for c in range(N_CORES):
        sl = slice(c * BL, (c + 1) * BL)
        in_maps.append(
            {"dq8": np.ascontiguousarray(dq8[sl]), "init8": np.ascontiguousarray(init8[sl])}
        )
    res = run_bass_kernel_spmd(nc, in_maps, core_ids=list(range(N_CORES)))
    out = np.empty((B, S, CH_OUT), dtype=np.float32)
    for c in range(N_CORES):
        out[c * BL : (c + 1) * BL] = res.results[c]["out"]
    return out


# revision 8
# speedup vs baseline: 1.4226x; 1.0532x over previous
"""Trainium2 Bass kernel for nn_DifferentiableBundleAdjustment.

Reference semantics (see problem):
  delta = dba_params[..., :7] * 0.1
  pos   = cumsum of delta[..., 0:3] along S, seeded with gt_state[:, 0, 0:3]
  quats = sequential q_t = normalize(q_{t-1} + delta[:, t-1, 3:7]), seeded
          with gt_state[:, 0, 3:7]
  out[..., 0:3] = pos, out[..., 3:7] = quats, out[..., 7:15] = 0

Sharding: pure data parallel over B=4096 across 8 cores (512 samples/core).
Per core:
  - 512 samples = 4 groups of 128; partition dim = sample-within-group.
  - pos: DVE tensor_tensor_scan (cumsum along free dim) per (group, comp).
  - quats: two software-pipelined chains (2 groups each), per step:
      u = 0.1*dq + q   (DVE scalar_tensor_tensor)
      n2 = sum(u^2)    (DVE tensor_mul + tensor_reduce)
      n = sqrt(n2)     (ACT)
      r = 1/n          (DVE reciprocal)
      q' = u*r         (DVE tensor_mul, broadcast r) -> written into outbuf
  - zeros: POOL memset of channels 7:15.
  - IO: contiguous chunked DMAs; output assembled in one SBUF buffer laid
    out exactly like out[b, s, ch] so the store is contiguous per partition.

This walrus build accepts at most one sync-wait per instruction; a
post-trace pass splits multi-wait instructions into single-wait NoOps.
"""

import numpy as np

import concourse.bass as bass
import concourse.mybir as mybir
from concourse.bass_utils import run_bass_kernel_spmd
from concourse.tile import TileContext

B, S, CH_IN, CH_OUT = 4096, 128, 32, 15
N_CORES = 8
BL = B // N_CORES  # 512 samples per core
G = BL // 128  # 4 groups
P = 128
F32 = mybir.dt.float32
Alu = mybir.AluOpType
X = mybir.AxisListType.X

_cache = {}


def _split_multi_waits(nc):
    """Walrus here allows only one sync-wait per instruction; hoist extras
    onto same-engine NoOps."""
    n = 0
    for fn in nc.m.functions:
        for blk in fn.blocks:
            out = []
            changed = False
            for inst in blk.instructions:
                si = getattr(inst, "sync_info", None)
                waits = list(si.on_wait) if si is not None and si.on_wait else []
                if len(waits) > 1:
                    for w in waits[:-1]:
                        nop = mybir.InstNoOp(name=f"{inst.name}-ws{n}")
                        nop.engine = inst.engine
                        nop.sync_info = mybir.SyncInfo(on_wait=[w], on_update=[])
                        out.append(nop)
                        n += 1
                    si.on_wait = [waits[-1]]
                    changed = True
                out.append(inst)
            if changed:
                blk.instructions = out
    return n


def _build():
    """One SPMD program; every core runs the same kernel on its slice."""
    nc = bass.Bass()
    dq_d = nc.dram_tensor("dq8", [BL, S, 8], F32, kind="ExternalInput")
    init_d = nc.dram_tensor("init8", [BL, 8], F32, kind="ExternalInput")
    out_d = nc.dram_tensor("out", [BL, S, CH_OUT], F32, kind="ExternalOutput")

    NCHUNK = 4  # input staging chunks along S
    SC = S // NCHUNK
    NOUT = 4  # output chunks along S
    OC = S // NOUT

    with TileContext(nc) as tc:
        with tc.tile_pool(name="pool", bufs=1) as pool, tc.tile_pool(
            name="w", bufs=6
        ) as wp:
            dq_t = pool.tile([P, G, S, 8], F32)
            outb = pool.tile([P, G, S, CH_OUT], F32)
            init_t = pool.tile([P, G, 8], F32)

            dq_r = dq_d.rearrange("(g p) s c -> p g s c", g=G)
            for k in range(NCHUNK):
                sl = slice(k * SC, (k + 1) * SC)
                nc.sync.dma_start(out=dq_t[:, :, sl, :], in_=dq_r[:, :, sl, :])
            nc.sync.dma_start(
                out=init_t, in_=init_d.rearrange("(g p) c -> p g c", g=G)
            )

            # zeros for channels 7:15 (POOL, off the critical path)
            nc.gpsimd.memset(outb[:, :, :, 7:15], 0.0)

            # init pos/quat into s=0
            nc.gpsimd.tensor_copy(out=outb[:, :, 0, 0:3], in_=init_t[:, :, 0:3])
            nc.vector.tensor_copy(out=outb[:, :, 0, 3:7], in_=init_t[:, :, 3:7])

            # --- positions: scaled cumsum via tensor_tensor_scan ------------
            # prescale pos deltas by 0.1 on POOL (in-place, strided)
            for k in range(NCHUNK):
                sl = slice(k * SC, (k + 1) * SC)
                nc.gpsimd.tensor_scalar_mul(
                    out=dq_t[:, :, sl, 0:3], in0=dq_t[:, :, sl, 0:3], scalar1=0.1
                )
            for g in range(G):
                for c in range(3):
                    nc.vector.tensor_tensor_scan(
                        out=outb[:, g, 1:S, c],
                        data0=dq_t[:, g, 0 : S - 1, c],
                        data1=dq_t[:, g, 0 : S - 1, c],
                        initial=init_t[:, g, c : c + 1],
                        op0=Alu.add,
                        op1=Alu.bypass,
                    )

            # --- quaternions: 4 chains (1 group each), pair-batched sqrt ----
            # chain state u_t kept unnormalized in ubuf; q_t = u_t * r_t is
            # reconstructed in a parallel postpass. Update is one fused STT:
            #   u_t = (u_{t-1} * r_{t-1}) + 0.1*dq_t  (dq prescaled by 0.1)
            # prescale quat deltas on POOL (pos cols already scaled above)
            for k in range(NCHUNK):
                sl = slice(k * SC, (k + 1) * SC)
                nc.gpsimd.tensor_scalar_mul(
                    out=dq_t[:, :, sl, 3:7], in0=dq_t[:, :, sl, 3:7], scalar1=0.1
                )

            # w-space recurrence: w_t = w_{t-1} + ||w_{t-1}|| * 0.1*dq_t, with
            # w_1 = q0 + 0.1*dq_1. Then q_t = w_t / ||w_t|| exactly. The serial
            # path per step is upd (STT) -> sum-of-squares (STT accum) ->
            # sqrt (ACT); no reciprocal in the loop.
            wbuf = [pool.tile([P, S, 4], F32, name=f"wbuf{g}") for g in range(G)]
            PAIRS = ((0, 1), (2, 3))
            # exact norms m_t = ||w_t||, one buffer per pair: [P, S, 2]
            nbufp = [pool.tile([P, S, 2], F32, name=f"nbufp{pi}") for pi in range(2)]

            def p1(pi, t):
                m2 = wp.tile([P, 2], F32, name=f"m2p{pi}")
                for a, g in enumerate(PAIRS[pi]):
                    if t == 1:
                        nc.vector.scalar_tensor_tensor(
                            out=wbuf[g][:, 1, :],
                            in0=init_t[:, g, 3:7],
                            scalar=1.0,
                            in1=dq_t[:, g, 0, 3:7],
                            op0=Alu.mult,
                            op1=Alu.add,
                        )
                    else:
                        nc.vector.scalar_tensor_tensor(
                            out=wbuf[g][:, t, :],
                            in0=dq_t[:, g, t - 1, 3:7],
                            scalar=nbufp[pi][:, t - 1, a : a + 1],
                            in1=wbuf[g][:, t - 1, :],
                            op0=Alu.mult,
                            op1=Alu.add,
                        )
                for a, g in enumerate(PAIRS[pi]):
                    sq = wp.tile([P, 4], F32, name=f"sqp{pi}{a}")
                    nc.vector.scalar_tensor_tensor(
                        out=sq,
                        in0=wbuf[g][:, t, :],
                        scalar=1.0,
                        in1=wbuf[g][:, t, :],
                        op0=Alu.mult,
                        op1=Alu.mult,
                        accum_out=m2[:, a : a + 1],
                    )
                nc.scalar.sqrt(out=nbufp[pi][:, t, :], in_=m2)

            # postpass chunk: q_t = w_t / m_t for s in [k*OC, (k+1)*OC), then
            # store those rows. Interleaved into the chain so output DMA
            # overlaps the remaining serial steps.
            rp = [pool.tile([P, S, 2], F32, name=f"rp{pi}") for pi in range(2)]
            out_r = out_d.rearrange("(g p) s c -> p g s c", g=G)

            def flush_chunk(k):
                s0, s1 = k * OC, (k + 1) * OC
                sq_ = slice(max(1, s0), s1)
                for pi in range(2):
                    nc.vector.reciprocal(
                        out=rp[pi][:, sq_, :], in_=nbufp[pi][:, sq_, :]
                    )
                for g in range(G):
                    pi, a = (0, g) if g < 2 else (1, g - 2)
                    rb = rp[pi][:, sq_, a : a + 1].broadcast_to(
                        [P, sq_.stop - sq_.start, 4]
                    )
                    nc.vector.tensor_mul(
                        out=outb[:, g, sq_, 3:7], in0=wbuf[g][:, sq_, :], in1=rb
                    )
                sl = slice(s0, s1)
                for g in range(G):
                    nc.sync.dma_start(
                        out=out_r[:, g, sl, :], in_=outb[:, g, sl, :]
                    )

            p1(0, 1)
            for t in range(1, S):
                p1(1, t)
                if t + 1 < S:
                    p1(0, t + 1)
                if (t + 1) % OC == 0 and t + 1 < S:
                    flush_chunk((t + 1) // OC - 1)
            flush_chunk(NOUT - 1)
    _split_multi_waits(nc)
    return nc


def kernel(dba_params, imu_measurements=None, gt_state=None, **_):
    dba_params = np.asarray(dba_params)
    gt_state = np.asarray(gt_state)
    assert dba_params.shape == (B, S, CH_IN), dba_params.shape

    if "nc" not in _cache:
        _cache["nc"] = _build()
    nc = _cache["nc"]

    dq8 = np.ascontiguousarray(dba_params[:, :, 0:8], dtype=np.float32)
    init8 = np.zeros((B, 8), dtype=np.float32)
    init8[:, 0:7] = gt_state[:, 0, 0:7]

    in_maps = []
    for c in range(N_CORES):
        sl = slice(c * BL, (c + 1) * BL)
        in_maps.append(
            {"dq8": np.ascontiguousarray(dq8[sl]), "init8": np.ascontiguousarray(init8[sl])}
        )
    res = run_bass_kernel_spmd(nc, in_maps, core_ids=list(range(N_CORES)))
    out = np.empty((B, S, CH_OUT), dtype=np.float32)
    for c in range(N_CORES):
        out[c * BL : (c + 1) * BL] = res.results[c]["out"]
    return out
